# revision 1
# baseline (speedup 1.0000x reference)
"""Trainium2 Bass kernel for nn_MetricSelfAttention.

Reference computation (B=4, W=2048, C=1024, N=16 heads, K=64):
    metric_n = P_n @ P_n^T                  (per-head bilinear form)
    proj = X @ W_proj^T ; split into per-head Q_n [W, K]
    S_n = tril(Q_n M_n Q_n^T) / sqrt(K)     (multiplicative causal mask, no softmax)
    U_n = S_n @ Q_n
    out = concat_n(U_n @ T_n) @ W_mixer^T

Device algorithm (per core; 8 cores = 4 batches x 2 head-groups of 8 heads):
  Host folds:  M'_n = P_n P_n^T / sqrt(K),  Wm2_n = T_n @ W_mixer[:, nK:(n+1)K]^T
  so that out_partial = sum_n U_n @ Wm2_n with U_n = tril(Q_n M'_n Q_n^T) @ Q_n.

  Causal decomposition (block row i of 128):
    U_i = A_i @ KV_i + tril(A_i Q_i^T) @ Q_i,   A = Q M',  KV_i = sum_{j<i} Q_j^T Q_j
  which turns the O(W^2 K) masked product into O(W K^2) prefix work plus
  O(W * 128 * K) diagonal blocks -- a ~4.5x FLOP reduction vs dense-causal.

  Heads are processed in pairs stacked on the partition dim (2x64=128) to keep
  the PE array full.  All matmuls run in bf16 with fp32 PSUM accumulation.
"""

import os
import sys

import numpy as np
import ml_dtypes

if "/opt/trn_rl_repo" not in sys.path:
    sys.path.insert(0, "/opt/trn_rl_repo")

import concourse.bass as bass
import concourse.tile as tile
from concourse import bacc, mybir
from concourse.bass_utils import run_bass_kernel_spmd

BF16 = mybir.dt.bfloat16
F32 = mybir.dt.float32

B, W, C, NHEADS, K = 4, 2048, 1024, 16, 64
HPG = 8          # heads per group (per core)
NPAIR = 4        # head pairs per core
GK = HPG * K     # 512: head-group projection width

_NC_CACHE = {}
LAST_RESULTS = None  # for test.py introspection (exec_time_ns etc.)


def build_nc(w=W, mm_dt=BF16):
    """Build the per-core Bass program. Parameterized by sequence length for
    small-scale simulator testing."""
    nw = w // 128           # number of 128-row w-tiles
    csub = C // 128         # 8 contraction subtiles for the projections
    chunk = min(512, w)
    nch = w // chunk        # 512-wide chunks of the sequence dim

    nc = bacc.Bacc()
    xt_d = nc.declare_dram_parameter("xt", [C, w], mm_dt, isOutput=False)
    wpt_d = nc.declare_dram_parameter("wpt", [C, GK], mm_dt, isOutput=False)
    mblk_d = nc.declare_dram_parameter("mblk", [NPAIR, 128, 128], mm_dt, isOutput=False)
    wm2_d = nc.declare_dram_parameter("wm2", [NPAIR, 128, C], mm_dt, isOutput=False)
    triu2_d = nc.declare_dram_parameter("triu2", [128, 256], F32, isOutput=False)
    blkd_d = nc.declare_dram_parameter("blkd", [128, 128], F32, isOutput=False)
    out_d = nc.declare_dram_parameter("out", [w, C], F32, isOutput=True)

    from contextlib import ExitStack

    with tile.TileContext(nc) as tc, ExitStack() as ctx:
        const = ctx.enter_context(tc.tile_pool(name="const", bufs=1))
        persist = ctx.enter_context(tc.tile_pool(name="persist", bufs=1))

        # ---- constant / input loads ----
        # Descriptor issue on one sequencer is ~615ns/op, so split issue
        # between sync (xt) and gpsimd (weights).  Chunk-major xt order lets
        # phase A start after the first w-chunk of every c-subtile.
        wpt_sb = []
        for s in range(csub):
            t = const.tile([128, GK], mm_dt, name=f"wpt{s}", tag=f"wpt{s}")
            nc.gpsimd.dma_start(t[:], wpt_d[128 * s:128 * (s + 1), :])
            wpt_sb.append(t)
        xt_sb = [const.tile([128, w], mm_dt, name=f"xt{s}", tag=f"xt{s}")
                 for s in range(csub)]
        for ch in range(nch):
            for s in range(csub):
                nc.sync.dma_start(
                    xt_sb[s][:, chunk * ch:chunk * (ch + 1)],
                    xt_d[128 * s:128 * (s + 1), chunk * ch:chunk * (ch + 1)],
                )
        mblk_sb = const.tile([128, NPAIR * 128], mm_dt, name="mblk", tag="mblk")
        for p in range(NPAIR):
            nc.gpsimd.dma_start(mblk_sb[:, 128 * p:128 * (p + 1)], mblk_d[p])
        wm2_sb = []
        for p in range(NPAIR):
            t = const.tile([128, C], mm_dt, name=f"wm2_{p}", tag=f"wm2_{p}")
            nc.gpsimd.dma_start(t[:], wm2_d[p])
            wm2_sb.append(t)
        triu2_sb = const.tile([128, 256], F32, name="triu2", tag="triu2")
        nc.gpsimd.dma_start(triu2_sb[:], triu2_d[:])
        blkd_sb = const.tile([128, 128], F32, name="blkd", tag="blkd")
        nc.gpsimd.dma_start(blkd_sb[:], blkd_d[:])

        # ---- persistent intermediates (bf16) ----
        # q_nat: natural layout [w, k] -- w-tile i occupies cols [512i, 512i+512),
        #        inside which head h (0..7) owns cols [64h, 64h+64).
        q_nat = persist.tile([128, nw * GK], mm_dt, name="q_nat", tag="q_nat")
        # qt/at: transposed layout per pair p: cols [p*w, (p+1)*w); partitions
        #        0-63 = head 2p's K dims, 64-127 = head 2p+1's.
        qt_sb = persist.tile([128, NPAIR * w], mm_dt, name="qt_sb", tag="qt_sb")
        at_sb = persist.tile([128, NPAIR * w], mm_dt, name="at_sb", tag="at_sb")
        # per-i blockdiag(KV_a, KV_b) lhsT tiles for the U-main matmuls
        kv_sb = persist.tile([128, NPAIR * nw * 128], mm_dt, name="kv_sb", tag="kv_sb")

        # ---- phase A: natural projection  Q[wtile] = XT[:,wtile]^T @ WpT ----
        with tc.tile_pool(name="psA", bufs=4, space="PSUM") as psA:
            for i in range(nw):
                ps = psA.tile([128, GK], F32, name="projnat", tag="projnat")
                for s in range(csub):
                    nc.tensor.matmul(
                        ps[:],
                        lhsT=xt_sb[s][:, 128 * i:128 * (i + 1)],
                        rhs=wpt_sb[s][:],
                        start=(s == 0),
                        stop=(s == csub - 1),
                    )
                nc.vector.tensor_copy(q_nat[:, GK * i:GK * (i + 1)], ps[:])

            # ---- phase B: transposed projection per pair:
            #      QT_pair[:, chunk] = WpT[:, pair]^T @ XT[:, chunk] ----
            for p in range(NPAIR):
                for ch in range(nch):
                    ps = psA.tile([128, chunk], F32, name="qtps", tag="projnat")
                    for s in range(csub):
                        nc.tensor.matmul(
                            ps[:],
                            lhsT=wpt_sb[s][:, 128 * p:128 * (p + 1)],
                            rhs=xt_sb[s][:, chunk * ch:chunk * (ch + 1)],
                            start=(s == 0),
                            stop=(s == csub - 1),
                        )
                    nc.vector.tensor_copy(
                        qt_sb[:, p * w + chunk * ch:p * w + chunk * (ch + 1)], ps[:]
                    )
            # ---- phase C: AT_pair = blockdiag(M'a, M'b) @ QT_pair ----
            for p in range(NPAIR):
                for ch in range(nch):
                    ps = psA.tile([128, chunk], F32, name="atps", tag="projnat")
                    nc.tensor.matmul(
                        ps[:],
                        lhsT=mblk_sb[:, 128 * p:128 * (p + 1)],
                        rhs=qt_sb[:, p * w + chunk * ch:p * w + chunk * (ch + 1)],
                        start=True,
                        stop=True,
                    )
                    nc.vector.tensor_copy(
                        at_sb[:, p * w + chunk * ch:p * w + chunk * (ch + 1)], ps[:]
                    )

        # ---- phase D: st_all / ut_all hold every (i, pair) block in SBUF ----
        st_all = persist.tile([128, nw * NPAIR * 256], mm_dt, name="st_all",
                              tag="st_all")
        ut_all = persist.tile([128, nw * NPAIR * 128], mm_dt, name="ut_all",
                              tag="ut_all")
        gram_sb = persist.tile([128, NPAIR * 128], F32, name="gram_sb",
                               tag="gram_sb")
        nc.vector.memset(gram_sb[:], 0.0)

        # Scope 2 -- D0 emitted alongside D1a (independent PE streams).
        # Gram terms drain PE->PSUM->SBUF(bf16) immediately (ACT copies), so
        # PSUM slots recycle without waiting on the serial prefix chain; the
        # prefix sum + blockdiag mask then run purely in SBUF (DVE/GpSimd),
        # entirely off the PE critical path.
        gt_all = persist.tile([128, max(nw - 1, 1) * NPAIR * 128], mm_dt,
                              name="gt_all", tag="gt_all")
        with tc.tile_pool(name="gtermp", bufs=3, space="PSUM") as gterm_pool, \
                tc.tile_pool(name="stp", bufs=5, space="PSUM") as st_pool:
            for i in range(nw - 1):  # last block's gram term is never consumed
                gterm = gterm_pool.tile([128, NPAIR * 128], F32, name="gterm",
                                        tag="gterm")
                for p in range(NPAIR):
                    qp = q_nat[:, GK * i + 128 * p:GK * i + 128 * (p + 1)]
                    nc.tensor.matmul(
                        gterm[:, 128 * p:128 * (p + 1)],
                        lhsT=qp, rhs=qp,
                        start=(p == 0),
                        stop=(p == NPAIR - 1),
                    )
                nc.scalar.copy(
                    gt_all[:, i * NPAIR * 128:(i + 1) * NPAIR * 128], gterm[:]
                )
            # prefix chain in SBUF: gram += term_{i-1}; kv_i = blkdiag(gram)
            for i in range(1, nw):
                nc.vector.tensor_add(
                    gram_sb[:], gram_sb[:],
                    gt_all[:, (i - 1) * NPAIR * 128:i * NPAIR * 128],
                )
                for p in range(NPAIR):
                    nc.gpsimd.tensor_mul(
                        kv_sb[:, (p * nw + i) * 128:(p * nw + i) * 128 + 128],
                        gram_sb[:, 128 * p:128 * (p + 1)], blkd_sb[:],
                    )

            # D1a: all diagonal blocks S_ii^T = Q_i @ A_i^T (row-tiled pairs;
            # the two concurrent row-group matmuls MUST hit different PSUM
            # banks -- same-bank concurrent PE writes crash the device),
            # triu-masked on the way to SBUF.
            for i in range(nw):
                for p in range(NPAIR):
                    st0 = (i * NPAIR + p) * 256
                    for h in range(2):
                        stp = st_pool.tile([128, 128], F32, name="st", tag="st")
                        nc.tensor.matmul(
                            stp[:],
                            lhsT=qt_sb[64 * h:64 * (h + 1), p * w + 128 * i:p * w + 128 * (i + 1)],
                            rhs=at_sb[64 * h:64 * (h + 1), p * w + 128 * i:p * w + 128 * (i + 1)],
                            start=True,
                            stop=True,
                        )
                        nc.vector.tensor_mul(
                            st_all[:, st0 + 128 * h:st0 + 128 * (h + 1)], stp[:],
                            triu2_sb[:, 0:128],
                        )

        # Scope 3 -- D1b (UT assembly) and D1c (mixer) interleaved per w-tile
        # so the mixer PE work pipelines behind the UT copies.
        with tc.tile_pool(name="utp", bufs=4, space="PSUM") as ut_pool, \
                tc.tile_pool(name="mixp", bufs=4, space="PSUM") as mix_pool, \
                tc.tile_pool(name="outp", bufs=3) as outp:
            for i in range(nw):
                # UT_pair_i [128(k-pair), 128(w)] = KV_i^T A_i^T + Q_i^T Smask_ii^T
                for p in range(NPAIR):
                    st0 = (i * NPAIR + p) * 256
                    utp = ut_pool.tile([128, 128], F32, name="ut", tag="ut")
                    if i > 0:
                        nc.tensor.matmul(
                            utp[:],
                            lhsT=kv_sb[:, (p * nw + i) * 128:(p * nw + i) * 128 + 128],
                            rhs=at_sb[:, p * w + 128 * i:p * w + 128 * (i + 1)],
                            start=True,
                            stop=False,
                            skip_group_check=True,
                        )
                    for h in range(2):
                        # partition-split accumulation group: the sim's flat
                        # zero-region bookkeeping can't express it (HW
                        # has_written bits are per partition), so skip the
                        # sim-side check
                        nc.tensor.matmul(
                            utp[64 * h:64 * (h + 1), :],
                            lhsT=q_nat[:, GK * i + 128 * p + 64 * h:GK * i + 128 * p + 64 * (h + 1)],
                            rhs=st_all[:, st0 + 128 * h:st0 + 128 * (h + 1)],
                            start=(i == 0),
                            stop=True,
                            skip_group_check=True,
                        )
                    nc.scalar.copy(
                        ut_all[:, (i * NPAIR + p) * 128:(i * NPAIR + p) * 128 + 128],
                        utp[:],
                    )

                # mixer: out[i-block] = sum_p UT_pair_i^T @ Wm2_pair
                # (p-outer would reuse lhsT across chunks, but accumulation
                # is over p, so keep cm-outer; PSUM accumulates per chunk)
                out_sb = outp.tile([128, C], F32, name="out_sb", tag="out_sb")
                for cm in range(C // 512):
                    mx = mix_pool.tile([128, 512], F32, name="mx", tag="mx")
                    for p in range(NPAIR):
                        nc.tensor.matmul(
                            mx[:],
                            lhsT=ut_all[:, (i * NPAIR + p) * 128:(i * NPAIR + p) * 128 + 128],
                            rhs=wm2_sb[p][:, 512 * cm:512 * (cm + 1)],
                            start=(p == 0),
                            stop=(p == NPAIR - 1),
                        )
                    nc.vector.tensor_copy(out_sb[:, 512 * cm:512 * (cm + 1)], mx[:])
                nc.sync.dma_start(out_d[128 * i:128 * (i + 1), :], out_sb[:])

    # Bacc defers register allocation + wait-splitting to finalize();
    # run_bass_via_pjrt serializes the module as-is, so finalize here.
    nc.finalize()
    return nc


def _get_nc(w=W):
    if w not in _NC_CACHE:
        _NC_CACHE[w] = build_nc(w)
    return _NC_CACHE[w]


def make_in_maps(x, wp, pm, tf, wm, w=W):
    """Host-side shard prep: per-core input dict list (cores c: b=c%4, g=c//4)."""
    bf = ml_dtypes.bfloat16
    metric = np.einsum("nij,nkj->nik", pm, pm) / np.sqrt(np.float32(K))
    # Wm2_n = T_n @ W_mixer[:, nK:(n+1)K]^T : [K, C]
    wm2 = np.stack([tf[n] @ wm[:, n * K:(n + 1) * K].T for n in range(NHEADS)])

    triu2 = np.zeros((128, 256), np.float32)
    tri = np.triu(np.ones((128, 128), np.float32))
    triu2[:, :128] = tri
    triu2[:, 128:] = tri
    blkd = np.zeros((128, 128), np.float32)
    blkd[:64, :64] = 1.0
    blkd[64:, 64:] = 1.0

    in_maps = []
    for c in range(8):
        b, g = c % 4, c // 4
        xt = np.ascontiguousarray(x[b][:w].T).astype(bf)                    # [C, w]
        wpt = np.ascontiguousarray(wp[GK * g:GK * (g + 1), :].T).astype(bf)  # [C, GK]
        mblk = np.zeros((NPAIR, 128, 128), np.float32)
        wm2c = np.zeros((NPAIR, 128, C), np.float32)
        for p in range(NPAIR):
            ha, hb = HPG * g + 2 * p, HPG * g + 2 * p + 1
            mblk[p, :64, :64] = metric[ha]
            mblk[p, 64:, 64:] = metric[hb]
            wm2c[p, :64, :] = wm2[ha]
            wm2c[p, 64:, :] = wm2[hb]
        in_maps.append({
            "xt": xt,
            "wpt": wpt,
            "mblk": mblk.astype(bf),
            "wm2": wm2c.astype(bf),
            "triu2": triu2,
            "blkd": blkd,
        })
    return in_maps


def _ensure_ntff_hook():
    """The agent image lacks antenv.axon_hooks; synthesize it and register the
    ctypes NTFF profile hook from trn_agent_boot so trace=True works."""
    try:
        from antenv.axon_hooks import get_axon_ntff_profile_hook  # noqa: F401
        return
    except ImportError:
        pass
    import types

    import antenv

    mod = types.ModuleType("antenv.axon_hooks")
    _box = {}
    mod.set_axon_ntff_profile_hook = lambda h: _box.__setitem__("h", h)
    mod.get_axon_ntff_profile_hook = lambda: _box.get("h")
    sys.modules["antenv.axon_hooks"] = mod
    antenv.axon_hooks = mod
    try:
        from trn_agent_boot.trn_boot import _ntff_profile_via_ctypes

        h = _ntff_profile_via_ctypes("/opt/axon/libaxon_pjrt.so")
        if h is not None:
            mod.set_axon_ntff_profile_hook(h)
    except Exception as e:  # profiling degrades, run still works
        print(f"ntff hook setup failed: {e}", file=sys.stderr)


def kernel(**inputs):
    global LAST_RESULTS
    x = np.asarray(inputs["in_sequence_bwc"], np.float32)
    wp = np.asarray(inputs["W_proj"], np.float32)
    pm = np.asarray(inputs["pre_metric_nkk"], np.float32)
    tf = np.asarray(inputs["transforms_nkk"], np.float32)
    wm = np.asarray(inputs["W_mixer"], np.float32)

    in_maps = make_in_maps(x, wp, pm, tf, wm)
    nc = _get_nc()
    trace = bool(int(os.environ.get("KERNEL_TRACE", "0")))
    if trace:
        _ensure_ntff_hook()
    res = run_bass_kernel_spmd(nc, in_maps, list(range(8)), trace=trace)
    LAST_RESULTS = res
    outs = [r["out"] for r in res.results]
    full = np.empty((B, W, C), np.float32)
    for b in range(B):
        full[b] = outs[b] + outs[4 + b]
    return full



# revision 12
# speedup vs baseline: 1.4275x; 1.4275x over previous
"""Trainium2 Bass kernel for nn_MetricSelfAttention.

Reference computation (B=4, W=2048, C=1024, N=16 heads, K=64):
    metric_n = P_n @ P_n^T                  (per-head bilinear form)
    proj = X @ W_proj^T ; split into per-head Q_n [W, K]
    S_n = tril(Q_n M_n Q_n^T) / sqrt(K)     (multiplicative causal mask, no softmax)
    U_n = S_n @ Q_n
    out = concat_n(U_n @ T_n) @ W_mixer^T

Device algorithm (per core; 8 cores = 4 batches x 2 head-groups of 8 heads):
  Host folds:  M'_n = P_n P_n^T / sqrt(K),  Wm2_n = T_n @ W_mixer[:, nK:(n+1)K]^T
  so that out_partial = sum_n U_n @ Wm2_n with U_n = tril(Q_n M'_n Q_n^T) @ Q_n.

  Causal decomposition (block row i of 128):
    U_i = A_i @ KV_i + tril(A_i Q_i^T) @ Q_i,   A = Q M',  KV_i = sum_{j<i} Q_j^T Q_j

  v2 structure (vs the v1 baseline):
   - the transposed projection QT is obtained by PE transposes of the natural
     projection (8K PE rows) instead of a full second projection pass (64K rows)
   - PSUM drains are batched into [128,512] ops and spread across DVE/Scalar/
     GpSimd so the PE never waits on a drain
   - the KV prefix is 4 independent per-pair add chains over pre-masked gram
     terms (no serial add->blockdiag-mul chain on the PE critical path)
   - D1a/D1b/mixer are emitted interleaved per w-tile to keep the PE streaming
"""

import os
import sys

import numpy as np
import ml_dtypes

if "/opt/trn_rl_repo" not in sys.path:
    sys.path.insert(0, "/opt/trn_rl_repo")

import concourse.bass as bass
import concourse.tile as tile
from concourse import bacc, mybir
from concourse.bass_utils import run_bass_kernel_spmd

BF16 = mybir.dt.bfloat16
F32 = mybir.dt.float32

B, W, C, NHEADS, K = 4, 2048, 1024, 16, 64
HPG = 8          # heads per group (per core)
NPAIR = 4        # head pairs per core
GK = HPG * K     # 512: head-group projection width

_NC_CACHE = {}
LAST_RESULTS = None  # for test.py introspection (exec_time_ns etc.)


def build_nc(w=W, mm_dt=BF16):
    """Build the per-core Bass program. Parameterized by sequence length for
    small-scale simulator testing."""
    nw = w // 128           # number of 128-row w-tiles
    csub = C // 128         # 8 contraction subtiles for the projections
    chunk = min(512, w)
    nch = w // chunk        # 512-wide chunks of the sequence dim

    nc = bacc.Bacc()
    xt_d = nc.declare_dram_parameter("xt", [C, w], mm_dt, isOutput=False)
    wpt_d = nc.declare_dram_parameter("wpt", [C, GK], mm_dt, isOutput=False)
    mblk_d = nc.declare_dram_parameter("mblk", [NPAIR, 128, 128], mm_dt, isOutput=False)
    wm2_d = nc.declare_dram_parameter("wm2", [NPAIR, 128, C], mm_dt, isOutput=False)
    triu4_d = nc.declare_dram_parameter("triu4", [128, 512], F32, isOutput=False)
    blkd4_d = nc.declare_dram_parameter("blkd4", [128, 512], F32, isOutput=False)
    ident_d = nc.declare_dram_parameter("ident", [128, 128], mm_dt, isOutput=False)
    out_d = nc.declare_dram_parameter("out", [w, C], F32, isOutput=True)

    from contextlib import ExitStack

    with tile.TileContext(nc) as tc, ExitStack() as ctx:
        const = ctx.enter_context(tc.tile_pool(name="const", bufs=1))
        persist = ctx.enter_context(tc.tile_pool(name="persist", bufs=1))

        # ---- constant / input loads ----
        # Split descriptor issue between sync (xt) and gpsimd (weights).
        # Chunk-major xt order lets phase A start after the first w-chunk of
        # every c-subtile.
        wpt_sb = []
        for s in range(csub):
            t = const.tile([128, GK], mm_dt, name=f"wpt{s}", tag=f"wpt{s}")
            nc.gpsimd.dma_start(t[:], wpt_d[128 * s:128 * (s + 1), :])
            wpt_sb.append(t)
        xt_sb = [const.tile([128, w], mm_dt, name=f"xt{s}", tag=f"xt{s}")
                 for s in range(csub)]
        for ch in range(nch):
            for s in range(csub):
                nc.sync.dma_start(
                    xt_sb[s][:, chunk * ch:chunk * (ch + 1)],
                    xt_d[128 * s:128 * (s + 1), chunk * ch:chunk * (ch + 1)],
                )
        mblk_sb = const.tile([128, NPAIR * 128], mm_dt, name="mblk", tag="mblk")
        for p in range(NPAIR):
            nc.gpsimd.dma_start(mblk_sb[:, 128 * p:128 * (p + 1)], mblk_d[p])
        wm2_sb = []
        for p in range(NPAIR):
            t = const.tile([128, C], mm_dt, name=f"wm2_{p}", tag=f"wm2_{p}")
            nc.gpsimd.dma_start(t[:], wm2_d[p])
            wm2_sb.append(t)
        triu4_sb = const.tile([128, 512], F32, name="triu4", tag="triu4")
        nc.gpsimd.dma_start(triu4_sb[:], triu4_d[:])
        blkd4_sb = const.tile([128, 512], F32, name="blkd4", tag="blkd4")
        nc.gpsimd.dma_start(blkd4_sb[:], blkd4_d[:])
        ident_sb = const.tile([128, 128], mm_dt, name="ident", tag="ident")
        nc.gpsimd.dma_start(ident_sb[:], ident_d[:])

        # ---- persistent intermediates (bf16) ----
        # q_nat: natural layout [w, k] -- w-tile i occupies cols [512i, 512i+512),
        #        inside which head h (0..7) owns cols [64h, 64h+64).
        q_nat = persist.tile([128, nw * GK], mm_dt, name="q_nat", tag="q_nat")
        # qt/at: transposed layout per pair p: cols [p*w, (p+1)*w); partitions
        #        0-63 = head 2p's K dims, 64-127 = head 2p+1's.
        qt_sb = persist.tile([128, NPAIR * w], mm_dt, name="qt_sb", tag="qt_sb")
        at_sb = persist.tile([128, NPAIR * w], mm_dt, name="at_sb", tag="at_sb")
        # per-i blockdiag(KV_a, KV_b) lhsT tiles for the U-main matmuls;
        # layout [i][p]: col block (i*NPAIR+p)*128
        kv_sb = persist.tile([128, nw * NPAIR * 128], mm_dt, name="kv_sb",
                             tag="kv_sb")
        # pre-masked gram terms, layout [i][p] like a [128, 512] row per i
        gt_all = persist.tile([128, max(nw - 1, 1) * NPAIR * 128], mm_dt,
                              name="gt_all", tag="gt_all")
        st_all = persist.tile([128, nw * NPAIR * 256], mm_dt, name="st_all",
                              tag="st_all")
        ut_all = persist.tile([128, nw * NPAIR * 128], mm_dt, name="ut_all",
                              tag="ut_all")

        # ============ phase 1: projection + transposes + gram + C ============
        with tc.tile_pool(name="psA", bufs=3, space="PSUM") as psA, \
                tc.tile_pool(name="psT", bufs=2, space="PSUM") as psT, \
                tc.tile_pool(name="psG", bufs=2, space="PSUM") as psG:

            def emit_A(i):
                ps = psA.tile([128, GK], F32, name="projnat", tag="projnat")
                for s in range(csub):
                    nc.tensor.matmul(
                        ps[:],
                        lhsT=xt_sb[s][:, 128 * i:128 * (i + 1)],
                        rhs=wpt_sb[s][:],
                        start=(s == 0),
                        stop=(s == csub - 1),
                    )
                nc.vector.tensor_copy(q_nat[:, GK * i:GK * (i + 1)], ps[:])

            def emit_T(i):
                # transpose the 4 pair-blocks of q_nat tile i into one psum
                # tile, then one scalar copy into the strided qt_sb layout
                # ([128 part, pair (stride w), 128 w-cols])
                ps = psT.tile([128, 512], mm_dt, name="qtT", tag="qtT")
                for p in range(NPAIR):
                    nc.tensor.transpose(
                        ps[:, 128 * p:128 * (p + 1)],
                        q_nat[:, GK * i + 128 * p:GK * i + 128 * (p + 1)],
                        ident_sb[:],
                    )
                qt_view = qt_sb[:].rearrange(
                    "part (n wdim) -> part n wdim", n=NPAIR
                )[:, :, 128 * i:128 * (i + 1)]
                nc.scalar.copy(qt_view, ps[:])

            def emit_G(i):
                # gram term for w-tile i (pair-stacked), masked on drain
                ps = psG.tile([128, NPAIR * 128], F32, name="gterm", tag="gterm")
                for p in range(NPAIR):
                    qp = q_nat[:, GK * i + 128 * p:GK * i + 128 * (p + 1)]
                    nc.tensor.matmul(
                        ps[:, 128 * p:128 * (p + 1)],
                        lhsT=qp, rhs=qp,
                        start=(p == 0),
                        stop=(p == NPAIR - 1),
                    )
                # masked drain straight to bf16 gram term (kills the
                # serial blockdiag-mul chain); GpSimd can't touch PSUM,
                # so this lives on DVE
                nc.vector.tensor_mul(
                    gt_all[:, i * NPAIR * 128:(i + 1) * NPAIR * 128],
                    ps[:], blkd4_sb[:],
                )

            def emit_C(ch):
                for p in range(NPAIR):
                    ps = psA.tile([128, chunk], F32, name="atps", tag="projnat")
                    nc.tensor.matmul(
                        ps[:],
                        lhsT=mblk_sb[:, 128 * p:128 * (p + 1)],
                        rhs=qt_sb[:, p * w + chunk * ch:p * w + chunk * (ch + 1)],
                        start=True,
                        stop=True,
                    )
                    nc.vector.tensor_copy(
                        at_sb[:, p * w + chunk * ch:p * w + chunk * (ch + 1)],
                        ps[:],
                    )

            for i in range(nw):
                emit_A(i)
                if i >= 1:
                    emit_T(i - 1)
                    if i - 1 < nw - 1:
                        emit_G(i - 1)
                if i >= 1 and i % 4 == 0:
                    emit_C(i // 4 - 1)
            emit_T(nw - 1)
            emit_C(nch - 1)

        # ---- KV prefix: 4 independent per-pair bf16 add chains ----
        # kv[p, 0] unused (i=0 has no main term); zero not required.
        # All-SBUF work, so it all goes to GpSimd (which can't touch PSUM
        # and has nothing else to do), keeping DVE free for PSUM drains.
        chain_eng = [nc.gpsimd, nc.gpsimd, nc.gpsimd, nc.gpsimd]
        for p in range(NPAIR):
            eng = chain_eng[p]
            # kv[p,1] = gt[0,p]
            eng.tensor_copy(
                kv_sb[:, (p * nw + 1) * 128:(p * nw + 1) * 128 + 128],
                gt_all[:, 0 * NPAIR * 128 + 128 * p:0 * NPAIR * 128 + 128 * (p + 1)],
            )
            for i in range(2, nw):
                eng.tensor_add(
                    kv_sb[:, (p * nw + i) * 128:(p * nw + i) * 128 + 128],
                    kv_sb[:, (p * nw + i - 1) * 128:(p * nw + i - 1) * 128 + 128],
                    gt_all[:, (i - 1) * NPAIR * 128 + 128 * p:(i - 1) * NPAIR * 128 + 128 * (p + 1)],
                )

        # ============ phase 2/3: D1a + D1b + mixer, interleaved per i ========
        with tc.tile_pool(name="psS", bufs=4, space="PSUM") as psS, \
                tc.tile_pool(name="psU", bufs=2, space="PSUM") as psU, \
                tc.tile_pool(name="psM", bufs=2, space="PSUM") as psM, \
                tc.tile_pool(name="outp", bufs=3) as outp:

            def emit_D1a(i):
                # all 8 diagonal blocks S_ii^T = Q_i @ A_i^T of tile i.
                # The h=0 / h=1 matmuls of a pair use contraction row groups
                # [0:64) / [64:128) and can execute CONCURRENTLY in the PE
                # array, so they must land in different PSUM banks: batch by h
                # (tile h holds that row-group's block for all 4 pairs).
                for h in range(2):
                    ps = psS.tile([128, 512], F32, name="st", tag="st")
                    for p in range(NPAIR):
                        nc.tensor.matmul(
                            ps[:, 128 * p:128 * (p + 1)],
                            lhsT=qt_sb[64 * h:64 * (h + 1),
                                       p * w + 128 * i:p * w + 128 * (i + 1)],
                            rhs=at_sb[64 * h:64 * (h + 1),
                                      p * w + 128 * i:p * w + 128 * (i + 1)],
                            start=(p == 0),
                            stop=(p == NPAIR - 1),
                        )
                    # masked drain into the strided st_all layout
                    # (cols i*1024 + p*256 + h*128)
                    dst = st_all[:, i * NPAIR * 256:(i + 1) * NPAIR * 256].rearrange(
                        "part (p two) -> part p two", p=NPAIR
                    )[:, :, 128 * h:128 * (h + 1)]
                    nc.vector.tensor_mul(dst, ps[:], triu4_sb[:])

            def emit_D1b(i):
                # UT for all 4 pairs of tile i in one [128, 512] psum tile.
                # Zero regions are per-partition 2KB rows, so the full-width
                # kv matmuls carry the visible start/stop bookkeeping; the
                # partition-split diag matmuls are inexpressible to the sim's
                # flat group tracker (HW has_written bits are per partition)
                # and use skip_group_check, with start=True only on the first
                # write to each partition-row range (i==0, p==0).
                ps = psU.tile([128, 512], F32, name="ut", tag="ut")
                for p in range(NPAIR):
                    st0 = i * NPAIR * 256 + 256 * p
                    if i > 0:
                        nc.tensor.matmul(
                            ps[:, 128 * p:128 * (p + 1)],
                            lhsT=kv_sb[:, (p * nw + i) * 128:(p * nw + i) * 128 + 128],
                            rhs=at_sb[:, p * w + 128 * i:p * w + 128 * (i + 1)],
                            start=(p == 0),
                            stop=(p == NPAIR - 1),
                        )
                    for h in range(2):
                        nc.tensor.matmul(
                            ps[64 * h:64 * (h + 1), 128 * p:128 * (p + 1)],
                            lhsT=q_nat[:, GK * i + 128 * p + 64 * h:
                                       GK * i + 128 * p + 64 * (h + 1)],
                            rhs=st_all[:, st0 + 128 * h:st0 + 128 * (h + 1)],
                            start=(i == 0 and p == 0),
                            stop=(i == 0 and p == NPAIR - 1 and h == 1),
                            skip_group_check=True,
                        )
                nc.scalar.copy(
                    ut_all[:, i * NPAIR * 128:(i + 1) * NPAIR * 128], ps[:]
                )

            def emit_mixer(i):
                out_sb = outp.tile([128, C], F32, name="out_sb", tag="out_sb")
                for cm in range(C // 512):
                    mx = psM.tile([128, 512], F32, name="mx", tag="mx")
                    for p in range(NPAIR):
                        nc.tensor.matmul(
                            mx[:],
                            lhsT=ut_all[:, (i * NPAIR + p) * 128:
                                        (i * NPAIR + p) * 128 + 128],
                            rhs=wm2_sb[p][:, 512 * cm:512 * (cm + 1)],
                            start=(p == 0),
                            stop=(p == NPAIR - 1),
                        )
                    if cm == 0:
                        nc.vector.tensor_copy(out_sb[:, 512 * cm:512 * (cm + 1)], mx[:])
                    else:
                        nc.scalar.copy(out_sb[:, 512 * cm:512 * (cm + 1)], mx[:])
                nc.sync.dma_start(out_d[128 * i:128 * (i + 1), :], out_sb[:])

            # pipeline: D1a runs one tile ahead of D1b/mixer
            emit_D1a(0)
            for i in range(nw):
                if i + 1 < nw:
                    emit_D1a(i + 1)
                emit_D1b(i)
                emit_mixer(i)

    nc.finalize()
    return nc


def _get_nc(w=W):
    if w not in _NC_CACHE:
        _NC_CACHE[w] = build_nc(w)
    return _NC_CACHE[w]


def make_in_maps(x, wp, pm, tf, wm, w=W):
    """Host-side shard prep: per-core input dict list (cores c: b=c%4, g=c//4)."""
    bf = ml_dtypes.bfloat16
    metric = np.einsum("nij,nkj->nik", pm, pm) / np.sqrt(np.float32(K))
    # Wm2_n = T_n @ W_mixer[:, nK:(n+1)K]^T : [K, C]
    wm2 = np.stack([tf[n] @ wm[:, n * K:(n + 1) * K].T for n in range(NHEADS)])

    tri = np.triu(np.ones((128, 128), np.float32))
    triu4 = np.tile(tri, (1, 4)).astype(np.float32)
    blkd = np.zeros((128, 128), np.float32)
    blkd[:64, :64] = 1.0
    blkd[64:, 64:] = 1.0
    blkd4 = np.tile(blkd, (1, 4)).astype(np.float32)
    ident = np.eye(128, dtype=np.float32)

    in_maps = []
    for c in range(8):
        b, g = c % 4, c // 4
        xt = np.ascontiguousarray(x[b][:w].T).astype(bf)                    # [C, w]
        wpt = np.ascontiguousarray(wp[GK * g:GK * (g + 1), :].T).astype(bf)  # [C, GK]
        mblk = np.zeros((NPAIR, 128, 128), np.float32)
        wm2c = np.zeros((NPAIR, 128, C), np.float32)
        for p in range(NPAIR):
            ha, hb = HPG * g + 2 * p, HPG * g + 2 * p + 1
            mblk[p, :64, :64] = metric[ha]
            mblk[p, 64:, 64:] = metric[hb]
            wm2c[p, :64, :] = wm2[ha]
            wm2c[p, 64:, :] = wm2[hb]
        in_maps.append({
            "xt": xt,
            "wpt": wpt,
            "mblk": mblk.astype(bf),
            "wm2": wm2c.astype(bf),
            "triu4": triu4,
            "blkd4": blkd4,
            "ident": ident.astype(bf),
        })
    return in_maps


def _ensure_ntff_hook():
    """The agent image lacks antenv.axon_hooks; synthesize it and register the
    ctypes NTFF profile hook from trn_agent_boot so trace=True works."""
    try:
        from antenv.axon_hooks import get_axon_ntff_profile_hook  # noqa: F401
        return
    except ImportError:
        pass
    import types

    import antenv

    mod = types.ModuleType("antenv.axon_hooks")
    _box = {}
    mod.set_axon_ntff_profile_hook = lambda h: _box.__setitem__("h", h)
    mod.get_axon_ntff_profile_hook = lambda: _box.get("h")
    sys.modules["antenv.axon_hooks"] = mod
    antenv.axon_hooks = mod
    try:
        from trn_agent_boot.trn_boot import _ntff_profile_via_ctypes

        h = _ntff_profile_via_ctypes("/opt/axon/libaxon_pjrt.so")
        if h is not None:
            mod.set_axon_ntff_profile_hook(h)
    except Exception as e:  # profiling degrades, run still works
        print(f"ntff hook setup failed: {e}", file=sys.stderr)


def kernel(**inputs):
    global LAST_RESULTS
    x = np.asarray(inputs["in_sequence_bwc"], np.float32)
    wp = np.asarray(inputs["W_proj"], np.float32)
    pm = np.asarray(inputs["pre_metric_nkk"], np.float32)
    tf = np.asarray(inputs["transforms_nkk"], np.float32)
    wm = np.asarray(inputs["W_mixer"], np.float32)

    in_maps = make_in_maps(x, wp, pm, tf, wm)
    nc = _get_nc()
    trace = bool(int(os.environ.get("KERNEL_TRACE", "0")))
    if trace:
        _ensure_ntff_hook()
    res = run_bass_kernel_spmd(nc, in_maps, list(range(8)), trace=trace)
    LAST_RESULTS = res
    outs = [r["out"] for r in res.results]
    full = np.empty((B, W, C), np.float32)
    for b in range(B):
        full[b] = outs[b] + outs[4 + b]
    return full
